# revision 18
# baseline (speedup 1.0000x reference)
"""DiagonalLinear: y = x * w + b (elementwise over features).

x: (16384, 4096) f32, w/b: (4096,) f32.
Sharding: data-parallel over batch across 8 NeuronCores (2048 rows each),
weight/bias replicated.

Per core: HWDGE loads x row-tiles [128, 4096] on the SP ring, DVE computes
mul+add in place, HWDGE stores on the ACT ring. The x/y traffic (64 MiB)
saturates the 16-SDMA fabric (~425 GB/s), so the w/b replication stays off
it: one 32 KiB DMA brings w|b into a single SBUF partition, then a K=1
fp32 PE matmul against a ones row (bit-exact on TRN2: 1.0*w) broadcasts
them across all 128 partitions via PSUM, with ACT copying PSUM->SBUF.
The first mul then only waits on x-tile 0 plus the ACT copies of the
w-half, which land well before the first load completes.
"""

import numpy as np

import concourse.bacc as bacc
import concourse.bass as bass
import concourse.mybir as mybir
import concourse.tile as tile
from concourse.bass_utils import run_bass_kernel_spmd

N_CORES = 8
BATCH = 16384
D = 4096
ROWS_PER_CORE = BATCH // N_CORES  # 2048
P = 128

BUFS = 4
MM_N = 512  # one PSUM bank per matmul

_CACHE = {}


def build_nc(bufs=BUFS):
    nc = bacc.Bacc()
    f32 = mybir.dt.float32
    x = nc.dram_tensor("x", [ROWS_PER_CORE, D], f32, kind="ExternalInput")
    wb_in = nc.dram_tensor("wb", [1, 2 * D], f32, kind="ExternalInput")
    y = nc.dram_tensor("y", [ROWS_PER_CORE, D], f32, kind="ExternalOutput")

    n_tiles = ROWS_PER_CORE // P  # 16
    x_r = x.rearrange("(n p) d -> n p d", p=P)
    y_r = y.rearrange("(n p) d -> n p d", p=P)

    with tile.TileContext(nc) as tc:
        with (
            tc.tile_pool(name="consts", bufs=1) as cpool,
            tc.tile_pool(name="work", bufs=bufs) as pool,
            tc.tile_pool(name="psum", bufs=4, space="PSUM") as ppool,
        ):
            consts = cpool.tile([P, 2 * D], f32)  # [:, :D]=w, [:, D:]=b
            wsb = cpool.tile([1, 2 * D], f32)
            ones = cpool.tile([1, P], f32)
            wt = consts[:, 0:D]
            bt = consts[:, D : 2 * D]

            # 32 KiB wb row on the ACT ring (SP ring belongs to the x loads),
            # then PE-broadcast it across partitions chunk by chunk.
            nc.scalar.dma_start(wsb[:, :], wb_in[:, :])
            nc.gpsimd.memset(ones[:, :], 1.0)
            for k in range(2 * D // MM_N):
                pt = ppool.tile([P, MM_N], f32)
                nc.tensor.matmul(
                    pt[:, :],
                    ones[:, :],
                    wsb[:, k * MM_N : (k + 1) * MM_N],
                    start=True,
                    stop=True,
                )
                nc.scalar.copy(consts[:, k * MM_N : (k + 1) * MM_N], pt[:, :])

            for i in range(n_tiles):
                t = pool.tile([P, D], f32)
                nc.sync.dma_start(t[:, :], x_r[i])
                nc.vector.tensor_mul(t[:, :], t[:, :], wt)
                nc.vector.tensor_add(t[:, :], t[:, :], bt)
                nc.scalar.dma_start(y_r[i], t[:, :])
    nc.compile()
    return nc


def _get_nc():
    if "nc" not in _CACHE:
        _CACHE["nc"] = build_nc()
    return _CACHE["nc"]


def run(input, weight, bias, nc=None, **spmd_kwargs):
    if nc is None:
        nc = _get_nc()
    x = np.ascontiguousarray(input, dtype=np.float32)
    wb = np.ascontiguousarray(
        np.stack([np.asarray(weight), np.asarray(bias)]).astype(np.float32)
    ).reshape(1, 2 * D)
    in_maps = [
        {"x": x[c * ROWS_PER_CORE : (c + 1) * ROWS_PER_CORE], "wb": wb}
        for c in range(N_CORES)
    ]
    res = run_bass_kernel_spmd(nc, in_maps, core_ids=list(range(N_CORES)), **spmd_kwargs)
    out = np.concatenate([r["y"] for r in res.results], axis=0)
    return out, res


def kernel(input, weight, bias):
    out, _ = run(input, weight, bias)
    return out
